# revision 6
# baseline (speedup 1.0000x reference)
"""Trainium2 Bass kernel: Bahdanau-style additive attention (nn_Attention).

Reference computation (per batch b):
    q = queries[:, b, :] @ Wq.T          # (Tq, C)   C = N_ATT = 128
    k = keys[:, b, :] @ Wk.T             # (Tk, C)
    v = values[:, b, :] @ Wv.T           # (Tk, C)
    logits[q, k] = sum_c Wvec[c] * tanh(q[q, c] + k[k, c])
    attn = softmax_k(logits)             # (Tq, Tk)
    out  = (attn @ v).T                  # (C, Tq)
returns (out (B, C, Tq), attn (B, Tq, Tk))

Sharding: data-parallel over batch B=8, one batch per NeuronCore (8 cores).

Per-core dataflow (all layouts chosen so C=128 sits on partitions):
  - PE transposes inputs, computes qproj (C, Tq), kproj (C, Tk), vprojT (Tk, C)
  - VectorE: S[c, (k, q)] = qproj[c, q] + kproj[c, k]   (per-partition scalar add)
  - ScalarE: E = tanh(S) in one giant activation per k-group (output fp16)
  - PE: logits built directly in (q, k) layout: for each (k, q-chunk) one
    matmul with lhsT = E-slice (c, 128q) and rhs = Wvec (c, 1) -> PSUM column k
  - softmax over free dim k, DMA attn out, PE-transpose attn, final matmul
    out = vprojT.T @ attnT
"""

import numpy as np

import concourse.bacc as bacc
import concourse.bass as bass
import concourse.mybir as mybir
from concourse import masks, tile

F32 = mybir.dt.float32
F16 = mybir.dt.float16
AF = mybir.ActivationFunctionType
AX = mybir.AxisListType

# Problem shapes (hardcoded per the harness contract).
TQ, B, TK = 512, 8, 256
NMEL, NCTX, C = 80, 640, 128
P = 128
IC = NCTX // P  # 5 contraction chunks for the 640-dim projections
QN = TQ // P    # 4 query chunks
KN = TK // P    # 2 key chunks
G = 16          # k-group size per tanh mega-instruction
NG = TK // G
USE_MAX_SUB = False  # logits bounded by sum|Wvec| ~ 9, exp is safe in fp32
N_CORES = 8


def _emit(tc, nc, q_d, k_d, v_d, wq_d, wk_d, wv_d, wvec_d, out_d, attn_d):
    with (
        tc.tile_pool(name="const", bufs=1) as cpool,
        tc.tile_pool(name="spool", bufs=2) as spool,
        tc.tile_pool(name="epool", bufs=2) as epool,
        tc.tile_pool(name="smx", bufs=2) as smx,
        tc.tile_pool(name="psT", bufs=2, space=bass.MemorySpace.PSUM) as psT,
        tc.tile_pool(name="psP", bufs=2, space=bass.MemorySpace.PSUM) as psP,
        tc.tile_pool(name="psA", bufs=1, space=bass.MemorySpace.PSUM) as psA,
    ):
        ident = cpool.tile([P, P], F32)
        masks.make_identity(nc, ident[:])

        wvec = cpool.tile([P, 1], F16)
        nc.sync.dma_start(wvec[:], wvec_d[:])

        # ---------------- queries -> qproj (C, TQ) ----------------
        q_in = cpool.tile([P, QN, NMEL], F32)
        nc.sync.dma_start(q_in[:], q_d.ap().rearrange("(n p) i -> p n i", p=P))
        wq_in = cpool.tile([P, NMEL], F32)
        nc.sync.dma_start(wq_in[:], wq_d[:])

        qT = cpool.tile([NMEL, TQ], F32)
        for n in range(QN):
            tp_q = psT.tile([NMEL, P], F32, tag="tp", name=f"tp_q{n}")
            nc.tensor.transpose(tp_q[:], q_in[:, n, :], ident[:])
            nc.vector.tensor_copy(qT[:, n * P:(n + 1) * P], tp_q[:])
        wqT = cpool.tile([NMEL, C], F32)
        tp_wq = psT.tile([NMEL, P], F32, tag="tp")
        nc.tensor.transpose(tp_wq[:], wq_in[:], ident[:])
        nc.vector.tensor_copy(wqT[:], tp_wq[:])

        qproj_ps = psP.tile([C, TQ], F32, tag="pp", name="qproj_ps")
        nc.tensor.matmul(qproj_ps[:], wqT[:], qT[:], start=True, stop=True)
        qproj = cpool.tile([C, TQ], F32)
        nc.vector.tensor_copy(qproj[:], qproj_ps[:])

        # ---------------- keys -> kproj (C, TK) ----------------
        k_in = cpool.tile([P, KN, NCTX], F32)
        nc.sync.dma_start(k_in[:], k_d.ap().rearrange("(n p) i -> p n i", p=P))
        wk_in = cpool.tile([P, NCTX], F32)
        nc.sync.dma_start(wk_in[:], wk_d[:])

        kT = cpool.tile([P, IC, TK], F32)
        for ic in range(IC):
            for n in range(KN):
                tp_k = psT.tile([P, P], F32, tag="tp", name=f"tp_k{ic}_{n}")
                nc.tensor.transpose(tp_k[:], k_in[:, n, ic * P:(ic + 1) * P], ident[:])
                nc.vector.tensor_copy(kT[:, ic, n * P:(n + 1) * P], tp_k[:])
        wkT = cpool.tile([P, IC, C], F32)
        for ic in range(IC):
            tp_wk = psT.tile([P, P], F32, tag="tp", name=f"tp_wk{ic}")
            nc.tensor.transpose(tp_wk[:], wk_in[:, ic * P:(ic + 1) * P], ident[:])
            nc.vector.tensor_copy(wkT[:, ic, :], tp_wk[:])

        kproj_ps = psP.tile([C, TK], F32, tag="pp", name="kproj_ps")
        for ic in range(IC):
            nc.tensor.matmul(kproj_ps[:], wkT[:, ic, :], kT[:, ic, :],
                             start=(ic == 0), stop=(ic == IC - 1))
        kproj = cpool.tile([C, TK], F32)
        nc.vector.tensor_copy(kproj[:], kproj_ps[:])

        # ---------------- energy + logits ----------------
        # attn_ps0 holds q-chunks 0,1 ; attn_ps1 holds q-chunks 2,3
        attn_ps0 = psA.tile([P, 2 * TK], F32)
        attn_ps1 = psA.tile([P, 2 * TK], F32)

        for g in range(NG):
            S = spool.tile([P, G * TQ], F32, tag="S", name=f"S{g}")
            for j in range(G):
                k = g * G + j
                nc.vector.tensor_scalar_add(
                    S[:, j * TQ:(j + 1) * TQ], qproj[:], kproj[:, k:k + 1])
            E = epool.tile([P, G * TQ], F16, tag="E", name=f"E{g}")
            nc.scalar.activation(E[:], S[:], AF.Tanh)
            for j in range(G):
                k = g * G + j
                for qc in range(4):
                    ps = attn_ps0 if qc < 2 else attn_ps1
                    col = (qc % 2) * TK + k
                    nc.tensor.matmul(
                        ps[:, col:col + 1],
                        E[:, j * TQ + qc * P: j * TQ + (qc + 1) * P],
                        wvec[:], start=True, stop=True)

        # ---------------- values -> vprojT (TK, C) ----------------
        # emitted after the energy loop: no dependency from the loop, so the
        # scheduler fills PE/DVE idle slots with this work instead of delaying
        # the first energy groups.
        v_in = cpool.tile([P, KN, NCTX], F32)
        nc.sync.dma_start(v_in[:], v_d.ap().rearrange("(n p) i -> p n i", p=P))
        wv_in = cpool.tile([P, NCTX], F32)
        nc.sync.dma_start(wv_in[:], wv_d[:])

        vT = cpool.tile([P, IC, TK], F32)
        for ic in range(IC):
            for n in range(KN):
                tp_v = psT.tile([P, P], F32, tag="tp", name=f"tp_v{ic}_{n}")
                nc.tensor.transpose(tp_v[:], v_in[:, n, ic * P:(ic + 1) * P], ident[:])
                nc.vector.tensor_copy(vT[:, ic, n * P:(n + 1) * P], tp_v[:])
        wvT = cpool.tile([P, IC, C], F32)
        for ic in range(IC):
            tp_wv = psT.tile([P, P], F32, tag="tp", name=f"tp_wv{ic}")
            nc.tensor.transpose(tp_wv[:], wv_in[:, ic * P:(ic + 1) * P], ident[:])
            nc.vector.tensor_copy(wvT[:, ic, :], tp_wv[:])

        vprojT = cpool.tile([P, KN, C], F32)
        for kc in range(KN):
            vp_ps = psP.tile([P, C], F32, tag="pp", name=f"vp_ps{kc}")
            for ic in range(IC):
                nc.tensor.matmul(vp_ps[:], vT[:, ic, kc * P:(kc + 1) * P],
                                 wvT[:, ic, :], start=(ic == 0), stop=(ic == IC - 1))
            nc.vector.tensor_copy(vprojT[:, kc, :], vp_ps[:])

        # ---------------- softmax (over k) + attn out + transpose ----------------
        attnT = cpool.tile([P, KN, TQ], F32)
        for qc in range(4):
            ps = attn_ps0 if qc < 2 else attn_ps1
            asl = ps[:, (qc % 2) * TK:(qc % 2 + 1) * TK]
            ae = smx.tile([P, TK], F32, tag="ae", name=f"ae{qc}")
            if USE_MAX_SUB:
                nmx = smx.tile([P, 1], F32, tag="nmx", name=f"nmx{qc}")
                nc.vector.reduce_max(nmx[:], asl, axis=AX.X, negate=True)
                nc.scalar.activation(ae[:], asl, AF.Exp, bias=nmx[:])
            else:
                nc.scalar.activation(ae[:], asl, AF.Exp)
            sm = smx.tile([P, 1], F32, tag="sm", name=f"sm{qc}")
            nc.vector.reduce_sum(sm[:], ae[:], axis=AX.X)
            rc = smx.tile([P, 1], F32, tag="rc", name=f"rc{qc}")
            nc.vector.reciprocal(rc[:], sm[:])
            an = smx.tile([P, TK], F32, tag="an", name=f"an{qc}")
            nc.vector.tensor_scalar_mul(an[:], ae[:], rc[:])
            nc.sync.dma_start(attn_d[qc * P:(qc + 1) * P, :], an[:])
            for kc in range(KN):
                tp_a = psT.tile([P, P], F32, tag="tp", name=f"tp_a{qc}_{kc}")
                nc.tensor.transpose(tp_a[:], an[:, kc * P:(kc + 1) * P], ident[:])
                nc.vector.tensor_copy(attnT[:, kc, qc * P:(qc + 1) * P], tp_a[:])

        # ---------------- out = vprojT.T @ attnT  (C, TQ) ----------------
        out_ps = psP.tile([C, TQ], F32, tag="pp", name="out_ps")
        for kc in range(KN):
            nc.tensor.matmul(out_ps[:], vprojT[:, kc, :], attnT[:, kc, :],
                             start=(kc == 0), stop=(kc == KN - 1))
        out_sb = cpool.tile([C, TQ], F32)
        nc.vector.tensor_copy(out_sb[:], out_ps[:])
        nc.sync.dma_start(out_d[:], out_sb[:])


def build_nc(loop_n=None):
    nc = bacc.Bacc("TRN2", target_bir_lowering=False, debug=False,
                   enable_asserts=False, num_devices=N_CORES)
    q_d = nc.dram_tensor("queries", [TQ, NMEL], F32, kind="ExternalInput")
    k_d = nc.dram_tensor("keys", [TK, NCTX], F32, kind="ExternalInput")
    v_d = nc.dram_tensor("values", [TK, NCTX], F32, kind="ExternalInput")
    wq_d = nc.dram_tensor("Wq", [C, NMEL], F32, kind="ExternalInput")
    wk_d = nc.dram_tensor("Wk", [C, NCTX], F32, kind="ExternalInput")
    wv_d = nc.dram_tensor("Wv", [C, NCTX], F32, kind="ExternalInput")
    wvec_d = nc.dram_tensor("wvec16", [C, 1], F16, kind="ExternalInput")
    out_d = nc.dram_tensor("out", [C, TQ], F32, kind="ExternalOutput")
    attn_d = nc.dram_tensor("attn", [TQ, TK], F32, kind="ExternalOutput")

    ET = mybir.EngineType
    with tile.TileContext(nc) as tc:
        if loop_n is None:
            _emit(tc, nc, q_d, k_d, v_d, wq_d, wk_d, wv_d, wvec_d, out_d, attn_d)
        else:
            # timing mode: run the whole (idempotent) body loop_n times on-device
            with tc.For_i(0, loop_n, 1,
                          hint_engines=(ET.PE, ET.DVE, ET.Activation, ET.SP)):
                _emit(tc, nc, q_d, k_d, v_d, wq_d, wk_d, wv_d, wvec_d, out_d, attn_d)
    nc.compile()
    return nc


_NC = None


def _get_nc():
    global _NC
    if _NC is None:
        _NC = build_nc()
    return _NC


def make_in_maps(queries, keys, values, Wq, Wk, Wv, Wvec):
    wvec16 = np.ascontiguousarray(np.asarray(Wvec).astype(np.float16).reshape(C, 1))
    wq = np.ascontiguousarray(np.asarray(Wq, dtype=np.float32))
    wk = np.ascontiguousarray(np.asarray(Wk, dtype=np.float32))
    wv = np.ascontiguousarray(np.asarray(Wv, dtype=np.float32))
    in_maps = []
    for b in range(B):
        in_maps.append({
            "queries": np.ascontiguousarray(np.asarray(queries)[:, b, :], dtype=np.float32),
            "keys": np.ascontiguousarray(np.asarray(keys)[:, b, :], dtype=np.float32),
            "values": np.ascontiguousarray(np.asarray(values)[:, b, :], dtype=np.float32),
            "Wq": wq, "Wk": wk, "Wv": wv, "wvec16": wvec16,
        })
    return in_maps


def kernel(queries, keys, values, Wq, Wk, Wv, Wvec):
    from concourse.bass2jax import run_bass_via_pjrt
    nc = _get_nc()
    in_maps = make_in_maps(queries, keys, values, Wq, Wk, Wv, Wvec)
    results = run_bass_via_pjrt(nc, in_maps, n_cores=N_CORES)
    out = np.stack([results[b]["out"] for b in range(B)], axis=0)
    attn = np.stack([results[b]["attn"] for b in range(B)], axis=0)
    return out, attn


# revision 25
# speedup vs baseline: 1.2567x; 1.2567x over previous
"""Trainium2 Bass kernel: Bahdanau-style additive attention (nn_Attention).

Reference computation (per batch b):
    q = queries[:, b, :] @ Wq.T          # (Tq, C)   C = N_ATT = 128
    k = keys[:, b, :] @ Wk.T             # (Tk, C)
    v = values[:, b, :] @ Wv.T           # (Tk, C)
    logits[q, k] = sum_c Wvec[c] * tanh(q[q, c] + k[k, c])
    attn = softmax_k(logits)             # (Tq, Tk)
    out  = (attn @ v).T                  # (C, Tq)
returns (out (B, C, Tq), attn (B, Tq, Tk))

Sharding: data-parallel over batch B=8, one batch per NeuronCore (8 cores).

Per-core dataflow (all layouts chosen so C=128 sits on partitions):
  - PE transposes inputs, computes qproj (C, Tq), kproj (C, Tk), vprojT (Tk, C)
  - VectorE: S[c, (k, q)] = qproj[c, q] + kproj[c, k]   (per-partition scalar add)
  - ScalarE: E = tanh(S) in one giant activation per k-group (output fp16)
  - PE: logits built directly in (q, k) layout: for each (k, q-chunk) one
    matmul with lhsT = E-slice (c, 128q) and rhs = Wvec (c, 1) -> PSUM column k
  - softmax over free dim k, DMA attn out, PE-transpose attn, final matmul
    out = vprojT.T @ attnT
"""

import numpy as np

import concourse.bacc as bacc
import concourse.bass as bass
import concourse.mybir as mybir
from concourse import masks, tile

F32 = mybir.dt.float32
F16 = mybir.dt.float16
AF = mybir.ActivationFunctionType
AX = mybir.AxisListType

# Problem shapes (hardcoded per the harness contract).
TQ, B, TK = 512, 8, 256
NMEL, NCTX, C = 80, 640, 128
P = 128
IC = NCTX // P  # 5 contraction chunks for the 640-dim projections
QN = TQ // P    # 4 query chunks
KN = TK // P    # 2 key chunks
G = 16          # k-group size per tanh mega-instruction
NG = TK // G
USE_MAX_SUB = False  # logits bounded by sum|Wvec| ~ 9, exp is safe in fp32
VALUES_LATE = True   # emit values/vprojT path after the energy loop
ABLATE = set()       # timing-only ablations: {"adds", "tanh", "reduce"}
SCHEME = "v1"        # "v1" (per-k-column N=1 reduce) or "v4" (N=512 + gather)
POOL_ADDS = 6        # per 16-k group, how many adds go to GPSIMD (v4 only)
RAMP = [2, 2, 4, 8]  # leading group sizes before steady-state G
BANK_GROUPED = False  # reduce-MM emission order: per-bank runs vs interleaved
KEYS_SPLIT = True    # split keys DMA/kproj into halves for earlier first add
RAMP_DOWN = []  # trailing group sizes (ramp-down hurts: ACT is saturated)
N_CORES = 8


def _emit(tc, nc, q_d, k_d, v_d, wq_d, wk_d, wv_d, wvec_d, out_d, attn_d):
    with (
        tc.tile_pool(name="const", bufs=1) as cpool,
        tc.tile_pool(name="spool", bufs=2) as spool,
        tc.tile_pool(name="epool", bufs=2) as epool,
        tc.tile_pool(name="smx", bufs=2) as smx,
        tc.tile_pool(name="psT", bufs=2, space=bass.MemorySpace.PSUM) as psT,
        tc.tile_pool(name="psP", bufs=2, space=bass.MemorySpace.PSUM) as psP,
        tc.tile_pool(name="psA", bufs=1, space=bass.MemorySpace.PSUM) as psA,
    ):
        ident = cpool.tile([P, P], F32)
        masks.make_identity(nc, ident[:])

        wvec = cpool.tile([P, 1], F16)
        NO_DMA = bool(ABLATE) and "dma" in ABLATE
        if not NO_DMA:
            nc.sync.dma_start(wvec[:], wvec_d[:])
        else:
            nc.gpsimd.memset(wvec[:], 0.01)

        # ------- keys -> kproj (C, TK) -------
        k_in = cpool.tile([P, KN, NCTX], F32)
        wk_in = cpool.tile([P, NCTX], F32)
        k_src = k_d.ap().rearrange("(n p) i -> p n i", p=P)
        if not NO_DMA:
            if KEYS_SPLIT:
                nc.sync.dma_start(k_in[:, 0, :], k_src[:, 0, :])
                nc.sync.dma_start(wk_in[:], wk_d[:])
                nc.sync.dma_start(k_in[:, 1, :], k_src[:, 1, :])
            else:
                nc.sync.dma_start(k_in[:], k_src[:])
                nc.sync.dma_start(wk_in[:], wk_d[:])
        else:
            nc.gpsimd.memset(k_in[:], 0.01)
            nc.gpsimd.memset(wk_in[:], 0.01)

        kT = cpool.tile([P, IC, TK], F32)
        wkT = cpool.tile([P, IC, C], F32)
        kproj_ps = psP.tile([C, TK], F32, tag="pp", name="kproj_ps")
        kproj = cpool.tile([C, TK], F32)
        if KEYS_SPLIT:
            for ic in range(IC):
                tp_k = psT.tile([P, P], F32, tag="tp", name=f"tp_k{ic}_0")
                nc.tensor.transpose(tp_k[:], k_in[:, 0, ic * P:(ic + 1) * P], ident[:])
                nc.vector.tensor_copy(kT[:, ic, 0:P], tp_k[:])
                tp_wk = psT.tile([P, P], F32, tag="tp", name=f"tp_wk{ic}")
                nc.tensor.transpose(tp_wk[:], wk_in[:, ic * P:(ic + 1) * P], ident[:])
                nc.vector.tensor_copy(wkT[:, ic, :], tp_wk[:])
            for ic in range(IC):
                nc.tensor.matmul(kproj_ps[:, 0:P], wkT[:, ic, :], kT[:, ic, 0:P],
                                 start=(ic == 0), stop=(ic == IC - 1))
            nc.vector.tensor_copy(kproj[:, 0:P], kproj_ps[:, 0:P])
        else:
            for ic in range(IC):
                for n in range(KN):
                    tp_k = psT.tile([P, P], F32, tag="tp", name=f"tp_k{ic}_{n}")
                    nc.tensor.transpose(tp_k[:], k_in[:, n, ic * P:(ic + 1) * P], ident[:])
                    nc.vector.tensor_copy(kT[:, ic, n * P:(n + 1) * P], tp_k[:])
                tp_wk = psT.tile([P, P], F32, tag="tp", name=f"tp_wk{ic}")
                nc.tensor.transpose(tp_wk[:], wk_in[:, ic * P:(ic + 1) * P], ident[:])
                nc.vector.tensor_copy(wkT[:, ic, :], tp_wk[:])
            for ic in range(IC):
                nc.tensor.matmul(kproj_ps[:], wkT[:, ic, :], kT[:, ic, :],
                                 start=(ic == 0), stop=(ic == IC - 1))
            nc.vector.tensor_copy(kproj[:], kproj_ps[:])

        # ---------------- queries -> qproj (C, TQ) ----------------
        q_in = cpool.tile([P, QN, NMEL], F32)
        wq_in = cpool.tile([P, NMEL], F32)
        q_src = q_d.ap().rearrange("(n p) i -> p n i", p=P)
        if not NO_DMA:
            nc.sync.dma_start(q_in[:], q_src[:])
            nc.sync.dma_start(wq_in[:], wq_d[:])
        else:
            nc.gpsimd.memset(q_in[:], 0.01)
            nc.gpsimd.memset(wq_in[:], 0.01)

        qT = cpool.tile([NMEL, TQ], F32)
        for n in range(QN):
            tp_q = psT.tile([NMEL, P], F32, tag="tp", name=f"tp_q{n}")
            nc.tensor.transpose(tp_q[:], q_in[:, n, :], ident[:])
            nc.vector.tensor_copy(qT[:, n * P:(n + 1) * P], tp_q[:])
        wqT = cpool.tile([NMEL, C], F32)
        tp_wq = psT.tile([NMEL, P], F32, tag="tp")
        nc.tensor.transpose(tp_wq[:], wq_in[:], ident[:])
        nc.vector.tensor_copy(wqT[:], tp_wq[:])

        qproj_ps = psP.tile([C, TQ], F32, tag="pp", name="qproj_ps")
        nc.tensor.matmul(qproj_ps[:], wqT[:], qT[:], start=True, stop=True)
        qproj = cpool.tile([C, TQ], F32)
        nc.vector.tensor_copy(qproj[:], qproj_ps[:])

        if KEYS_SPLIT:
            # second half of kproj (keys n=1)
            for ic in range(IC):
                tp_k1 = psT.tile([P, P], F32, tag="tp", name=f"tp_k{ic}_1")
                nc.tensor.transpose(tp_k1[:], k_in[:, 1, ic * P:(ic + 1) * P], ident[:])
                nc.vector.tensor_copy(kT[:, ic, P:2 * P], tp_k1[:])
            for ic in range(IC):
                nc.tensor.matmul(kproj_ps[:, P:2 * P], wkT[:, ic, :],
                                 kT[:, ic, P:2 * P],
                                 start=(ic == 0), stop=(ic == IC - 1))
            nc.vector.tensor_copy(kproj[:, P:2 * P], kproj_ps[:, P:2 * P])

        def _values_path():
            # ---------------- values -> vprojT (TK, C) ----------------
            v_in = cpool.tile([P, KN, NCTX], F32, name="v_in")
            if not NO_DMA:
                nc.sync.dma_start(v_in[:], v_d.ap().rearrange("(n p) i -> p n i", p=P))
            else:
                nc.gpsimd.memset(v_in[:], 0.01)
            wv_in = cpool.tile([P, NCTX], F32, name="wv_in")
            if not NO_DMA:
                nc.sync.dma_start(wv_in[:], wv_d[:])
            else:
                nc.gpsimd.memset(wv_in[:], 0.01)

            vT = cpool.tile([P, IC, TK], F32, name="vT")
            for ic in range(IC):
                for n in range(KN):
                    tp_v = psT.tile([P, P], F32, tag="tp", name=f"tp_v{ic}_{n}")
                    nc.tensor.transpose(tp_v[:], v_in[:, n, ic * P:(ic + 1) * P], ident[:])
                    nc.vector.tensor_copy(vT[:, ic, n * P:(n + 1) * P], tp_v[:])
            wvT = cpool.tile([P, IC, C], F32, name="wvT")
            for ic in range(IC):
                tp_wv = psT.tile([P, P], F32, tag="tp", name=f"tp_wv{ic}")
                nc.tensor.transpose(tp_wv[:], wv_in[:, ic * P:(ic + 1) * P], ident[:])
                nc.vector.tensor_copy(wvT[:, ic, :], tp_wv[:])

            vprojT = cpool.tile([P, KN, C], F32, name="vprojT")
            for kc in range(KN):
                vp_ps = psP.tile([P, C], F32, tag="pp", name=f"vp_ps{kc}")
                for ic in range(IC):
                    nc.tensor.matmul(vp_ps[:], vT[:, ic, kc * P:(kc + 1) * P],
                                     wvT[:, ic, :], start=(ic == 0), stop=(ic == IC - 1))
                nc.vector.tensor_copy(vprojT[:, kc, :], vp_ps[:])
            return vprojT

        vprojT = None
        if not VALUES_LATE:
            vprojT = _values_path()

        # ---------------- energy + logits ----------------
        # attn_ps0 holds q-chunks 0,1 ; attn_ps1 holds q-chunks 2,3
        attn_ps0 = psA.tile([P, 2 * TK], F32)
        attn_ps1 = psA.tile([P, 2 * TK], F32)

        if ABLATE & {"adds", "tanh", "reduce"}:
            # timing-only mode: stages in ABLATE are removed from the per-group
            # loop; const tiles stand in for their outputs.
            S_const = cpool.tile([P, G * TQ], F32, name="S_const")
            for j in range(G):
                nc.vector.tensor_scalar_add(
                    S_const[:, j * TQ:(j + 1) * TQ], qproj[:], kproj[:, j:j + 1])
            E_const = cpool.tile([P, G * TQ], F16, name="E_const")
            nc.scalar.activation(E_const[:], S_const[:], AF.Tanh)
            if "reduce" in ABLATE:
                nc.vector.tensor_copy(attn_ps0[:], qproj[:])
                nc.vector.tensor_copy(attn_ps1[:], qproj[:])
        # ramped group sizes: small first groups so the first tanh fires early
        if ABLATE:
            group_sizes = [G] * NG
        else:
            group_sizes = (RAMP + [G] * ((TK - sum(RAMP) - sum(RAMP_DOWN)) // G)
                           + RAMP_DOWN)
            assert sum(group_sizes) == TK
        k_base = 0
        for g, gs in enumerate(group_sizes):
            if "adds" not in ABLATE:
                S = spool.tile([P, G * TQ], F32, tag="S", name=f"S{g}")
                for j in range(gs):
                    k = k_base + j
                    nc.vector.tensor_scalar_add(
                        S[:, j * TQ:(j + 1) * TQ], qproj[:], kproj[:, k:k + 1])
            else:
                S = S_const
            if "tanh" not in ABLATE:
                E = epool.tile([P, G * TQ], F16, tag="E", name=f"E{g}")
                nc.scalar.activation(E[:, 0:gs * TQ], S[:, 0:gs * TQ], AF.Tanh)
            else:
                E = E_const
            if "reduce" not in ABLATE:
                if BANK_GROUPED:
                    for qcp in range(2):
                        ps = attn_ps0 if qcp == 0 else attn_ps1
                        for j in range(gs):
                            k = k_base + j
                            for h in range(2):
                                qc = 2 * qcp + h
                                col = h * TK + k
                                nc.tensor.matmul(
                                    ps[:, col:col + 1],
                                    E[:, j * TQ + qc * P: j * TQ + (qc + 1) * P],
                                    wvec[:], start=True, stop=True)
                else:
                    for j in range(gs):
                        k = k_base + j
                        for qc in range(4):
                            ps = attn_ps0 if qc < 2 else attn_ps1
                            col = (qc % 2) * TK + k
                            nc.tensor.matmul(
                                ps[:, col:col + 1],
                                E[:, j * TQ + qc * P: j * TQ + (qc + 1) * P],
                                wvec[:], start=True, stop=True)
            k_base += gs

        if VALUES_LATE:
            vprojT = _values_path()

        # ---------------- softmax (over k) + attn out + transpose ----------------
        attnT = cpool.tile([P, KN, TQ], F32)
        for qc in range(4):
            ps = attn_ps0 if qc < 2 else attn_ps1
            asl = ps[:, (qc % 2) * TK:(qc % 2 + 1) * TK]
            ae = smx.tile([P, TK], F32, tag="ae", name=f"ae{qc}")
            if USE_MAX_SUB:
                nmx = smx.tile([P, 1], F32, tag="nmx", name=f"nmx{qc}")
                nc.vector.reduce_max(nmx[:], asl, axis=AX.X, negate=True)
                nc.scalar.activation(ae[:], asl, AF.Exp, bias=nmx[:])
            else:
                nc.scalar.activation(ae[:], asl, AF.Exp)
            sm = smx.tile([P, 1], F32, tag="sm", name=f"sm{qc}")
            nc.vector.reduce_sum(sm[:], ae[:], axis=AX.X)
            rc = smx.tile([P, 1], F32, tag="rc", name=f"rc{qc}")
            nc.vector.reciprocal(rc[:], sm[:])
            an = smx.tile([P, TK], F32, tag="an", name=f"an{qc}")
            nc.vector.tensor_scalar_mul(an[:], ae[:], rc[:])
            if not NO_DMA:
                nc.sync.dma_start(attn_d[qc * P:(qc + 1) * P, :], an[:])
            for kc in range(KN):
                tp_a = psT.tile([P, P], F32, tag="tp", name=f"tp_a{qc}_{kc}")
                nc.tensor.transpose(tp_a[:], an[:, kc * P:(kc + 1) * P], ident[:])
                nc.vector.tensor_copy(attnT[:, kc, qc * P:(qc + 1) * P], tp_a[:])

        # ---------------- out = vprojT.T @ attnT  (C, TQ) ----------------
        out_ps = psP.tile([C, TQ], F32, tag="pp", name="out_ps")
        for kc in range(KN):
            nc.tensor.matmul(out_ps[:], vprojT[:, kc, :], attnT[:, kc, :],
                             start=(kc == 0), stop=(kc == KN - 1))
        out_sb = cpool.tile([C, TQ], F32)
        nc.vector.tensor_copy(out_sb[:], out_ps[:])
        if not NO_DMA:
            nc.sync.dma_start(out_d[:], out_sb[:])


def _emit_v4(tc, nc, q_d, k_d, v_d, wq_d, wk_d, wv_d, wvec32_d, out_d, attn_d):
    """V4: N=512 reduce matmuls (4 k-rows per PSUM bank at partitions 32j),
    DVE bank->SBUF bulk copy, SBUF->SBUF DMA partition-gather to attnT (k, q),
    exp + matmul row-sums + reciprocal-broadcast normalization.
    Adds split between DVE and GPSIMD (Pool)."""
    with (
        tc.tile_pool(name="const", bufs=1) as cpool,
        tc.tile_pool(name="spool", bufs=2) as spool,
        tc.tile_pool(name="slabs", bufs=4) as slabs,
        tc.tile_pool(name="smx", bufs=2) as smx,
        tc.tile_pool(name="psT", bufs=2, space=bass.MemorySpace.PSUM) as psT,
        tc.tile_pool(name="psP", bufs=2, space=bass.MemorySpace.PSUM) as psP,
        tc.tile_pool(name="psR", bufs=2, space=bass.MemorySpace.PSUM) as psR,
    ):
        ident = cpool.tile([P, P], F32)
        masks.make_identity(nc, ident[:])

        wvec32 = cpool.tile([P, 32], F32)
        nc.sync.dma_start(wvec32[:], wvec32_d[:])
        ones_k = cpool.tile([P, 1], F32)
        nc.gpsimd.memset(ones_k[:], 1.0)
        ones_1 = cpool.tile([1, P], F32)
        nc.gpsimd.memset(ones_1[:], 1.0)

        # ---------------- queries -> qproj (C, TQ) ----------------
        q_in = cpool.tile([P, QN, NMEL], F32)
        nc.sync.dma_start(q_in[:], q_d.ap().rearrange("(n p) i -> p n i", p=P))
        wq_in = cpool.tile([P, NMEL], F32)
        nc.sync.dma_start(wq_in[:], wq_d[:])

        qT = cpool.tile([NMEL, TQ], F32)
        for n in range(QN):
            tp_q = psT.tile([NMEL, P], F32, tag="tp", name=f"tp_q{n}")
            nc.tensor.transpose(tp_q[:], q_in[:, n, :], ident[:])
            nc.vector.tensor_copy(qT[:, n * P:(n + 1) * P], tp_q[:])
        wqT = cpool.tile([NMEL, C], F32)
        tp_wq = psT.tile([NMEL, P], F32, tag="tp")
        nc.tensor.transpose(tp_wq[:], wq_in[:], ident[:])
        nc.vector.tensor_copy(wqT[:], tp_wq[:])

        qproj_ps = psP.tile([C, TQ], F32, tag="pp", name="qproj_ps")
        nc.tensor.matmul(qproj_ps[:], wqT[:], qT[:], start=True, stop=True)
        qproj = cpool.tile([C, TQ], F32)
        nc.vector.tensor_copy(qproj[:], qproj_ps[:])

        if KEYS_SPLIT:
            # second half of kproj (keys n=1)
            for ic in range(IC):
                tp_k1 = psT.tile([P, P], F32, tag="tp", name=f"tp_k{ic}_1")
                nc.tensor.transpose(tp_k1[:], k_in[:, 1, ic * P:(ic + 1) * P], ident[:])
                nc.vector.tensor_copy(kT[:, ic, P:2 * P], tp_k1[:])
            for ic in range(IC):
                nc.tensor.matmul(kproj_ps[:, P:2 * P], wkT[:, ic, :],
                                 kT[:, ic, P:2 * P],
                                 start=(ic == 0), stop=(ic == IC - 1))
            nc.vector.tensor_copy(kproj[:, P:2 * P], kproj_ps[:, P:2 * P])

        # ---------------- keys -> kproj (C, TK) ----------------
        k_in = cpool.tile([P, KN, NCTX], F32)
        nc.sync.dma_start(k_in[:], k_d.ap().rearrange("(n p) i -> p n i", p=P))
        wk_in = cpool.tile([P, NCTX], F32)
        nc.sync.dma_start(wk_in[:], wk_d[:])

        kT = cpool.tile([P, IC, TK], F32)
        for ic in range(IC):
            for n in range(KN):
                tp_k = psT.tile([P, P], F32, tag="tp", name=f"tp_k{ic}_{n}")
                nc.tensor.transpose(tp_k[:], k_in[:, n, ic * P:(ic + 1) * P], ident[:])
                nc.vector.tensor_copy(kT[:, ic, n * P:(n + 1) * P], tp_k[:])
        wkT = cpool.tile([P, IC, C], F32)
        for ic in range(IC):
            tp_wk = psT.tile([P, P], F32, tag="tp", name=f"tp_wk{ic}")
            nc.tensor.transpose(tp_wk[:], wk_in[:, ic * P:(ic + 1) * P], ident[:])
            nc.vector.tensor_copy(wkT[:, ic, :], tp_wk[:])

        kproj_ps = psP.tile([C, TK], F32, tag="pp", name="kproj_ps")
        for ic in range(IC):
            nc.tensor.matmul(kproj_ps[:], wkT[:, ic, :], kT[:, ic, :],
                             start=(ic == 0), stop=(ic == IC - 1))
        kproj = cpool.tile([C, TK], F32)
        nc.vector.tensor_copy(kproj[:], kproj_ps[:])

        def _values_path():
            v_in = cpool.tile([P, KN, NCTX], F32, name="v_in")
            nc.sync.dma_start(v_in[:], v_d.ap().rearrange("(n p) i -> p n i", p=P))
            wv_in = cpool.tile([P, NCTX], F32, name="wv_in")
            nc.sync.dma_start(wv_in[:], wv_d[:])

            vT = cpool.tile([P, IC, TK], F32, name="vT")
            for ic in range(IC):
                for n in range(KN):
                    tp_v = psT.tile([P, P], F32, tag="tp", name=f"tp_v{ic}_{n}")
                    nc.tensor.transpose(tp_v[:], v_in[:, n, ic * P:(ic + 1) * P], ident[:])
                    nc.vector.tensor_copy(vT[:, ic, n * P:(n + 1) * P], tp_v[:])
            wvT = cpool.tile([P, IC, C], F32, name="wvT")
            for ic in range(IC):
                tp_wv = psT.tile([P, P], F32, tag="tp", name=f"tp_wv{ic}")
                nc.tensor.transpose(tp_wv[:], wv_in[:, ic * P:(ic + 1) * P], ident[:])
                nc.vector.tensor_copy(wvT[:, ic, :], tp_wv[:])

            vprojT = cpool.tile([P, KN, C], F32, name="vprojT")
            for kc in range(KN):
                vp_ps = psP.tile([P, C], F32, tag="pp", name=f"vp_ps{kc}")
                for ic in range(IC):
                    nc.tensor.matmul(vp_ps[:], vT[:, ic, kc * P:(kc + 1) * P],
                                     wvT[:, ic, :], start=(ic == 0), stop=(ic == IC - 1))
                nc.vector.tensor_copy(vprojT[:, kc, :], vp_ps[:])
            return vprojT

        vprojT = None
        if not VALUES_LATE:
            vprojT = _values_path()

        # ---------------- energy + logits -> attnT_raw (k, q) ----------------
        attnT_raw = cpool.tile([P, KN, TQ], F32)
        for g in range(NG):
            S = spool.tile([P, G * TQ], F32, tag="S", name=f"S{g}")
            for j in range(G):
                k = g * G + j
                eng = nc.gpsimd if (j % G) < POOL_ADDS else nc.vector
                eng.tensor_scalar_add(
                    S[:, j * TQ:(j + 1) * TQ], qproj[:], kproj[:, k:k + 1])
            nc.scalar.activation(S[:], S[:], AF.Tanh)
            for b4 in range(G // 4):
                k0 = g * G + b4 * 4
                bank = psR.tile([P, TQ], F32, tag="bank", name=f"bank{k0}")
                for r in range(4):
                    j = b4 * 4 + r
                    nc.tensor.matmul(bank[32 * r:32 * r + 32, :], wvec32[:],
                                     S[:, j * TQ:(j + 1) * TQ], start=True, stop=True,
                                     tile_position=(0, 32 * r))
                slab = slabs.tile([P, TQ], F32, tag="slab", name=f"slab{k0}")
                nc.vector.tensor_copy(slab[:], bank[:])
                src = slab[:].rearrange("(a b) f -> a b f", b=32)[:, 0, :]
                nc.sync.dma_start(
                    attnT_raw[(k0 % P):(k0 % P) + 4, k0 // P, :], src)

        if VALUES_LATE:
            vprojT = _values_path()

        # ---------------- exp + softmax ----------------
        expT = cpool.tile([P, KN, TQ], F32)
        for kc in range(KN):
            nc.scalar.activation(expT[:, kc, :], attnT_raw[:, kc, :], AF.Exp)

        # attn output in (q, k): transpose exp'd logits, normalize rows
        for qc in range(4):
            aqk = smx.tile([P, TK], F32, tag="aqk", name=f"aqk{qc}")
            for kc in range(KN):
                tp_a = psT.tile([P, P], F32, tag="tp", name=f"tp_a{qc}_{kc}")
                nc.tensor.transpose(tp_a[:], expT[:, kc, qc * P:(qc + 1) * P], ident[:])
                nc.vector.tensor_copy(aqk[:, kc * P:(kc + 1) * P], tp_a[:])
            sm = smx.tile([P, 1], F32, tag="sm", name=f"sm{qc}")
            nc.vector.reduce_sum(sm[:], aqk[:], axis=AX.X)
            rc = smx.tile([P, 1], F32, tag="rc", name=f"rc{qc}")
            nc.vector.reciprocal(rc[:], sm[:])
            an = smx.tile([P, TK], F32, tag="an", name=f"an{qc}")
            nc.vector.tensor_scalar_mul(an[:], aqk[:], rc[:])
            nc.sync.dma_start(attn_d[qc * P:(qc + 1) * P, :], an[:])

        # ---------------- out = (vprojT.T @ expT) * recip(colsums) ----------------
        out_ps = psP.tile([C, TQ], F32, tag="pp", name="out_ps")
        for kc in range(KN):
            nc.tensor.matmul(out_ps[:], vprojT[:, kc, :], expT[:, kc, :],
                             start=(kc == 0), stop=(kc == KN - 1))
        sums_ps = psP.tile([1, TQ], F32, tag="sums", name="sums_ps", bufs=1)
        for kc in range(KN):
            nc.tensor.matmul(sums_ps[:], ones_k[:], expT[:, kc, :],
                             start=(kc == 0), stop=(kc == KN - 1))
        recip_row = cpool.tile([1, TQ], F32)
        nc.vector.reciprocal(recip_row[:], sums_ps[:])
        rb_ps = psP.tile([P, TQ], F32, tag="rb", name="rb_ps", bufs=1)
        nc.tensor.matmul(rb_ps[:], ones_1[:], recip_row[:], start=True, stop=True)
        rb_sb = cpool.tile([P, TQ], F32)
        nc.vector.tensor_copy(rb_sb[:], rb_ps[:])
        out_sb = cpool.tile([C, TQ], F32)
        nc.vector.tensor_mul(out_sb[:], out_ps[:], rb_sb[:])
        nc.sync.dma_start(out_d[:], out_sb[:])


def build_nc(loop_n=None):
    nc = bacc.Bacc("TRN2", target_bir_lowering=False, debug=False,
                   enable_asserts=False, num_devices=N_CORES)
    q_d = nc.dram_tensor("queries", [TQ, NMEL], F32, kind="ExternalInput")
    k_d = nc.dram_tensor("keys", [TK, NCTX], F32, kind="ExternalInput")
    v_d = nc.dram_tensor("values", [TK, NCTX], F32, kind="ExternalInput")
    wq_d = nc.dram_tensor("Wq", [C, NMEL], F32, kind="ExternalInput")
    wk_d = nc.dram_tensor("Wk", [C, NCTX], F32, kind="ExternalInput")
    wv_d = nc.dram_tensor("Wv", [C, NCTX], F32, kind="ExternalInput")
    if SCHEME == "v4":
        wvec_d = nc.dram_tensor("wvec32", [C, 32], F32, kind="ExternalInput")
    else:
        wvec_d = nc.dram_tensor("wvec16", [C, 1], F16, kind="ExternalInput")
    out_d = nc.dram_tensor("out", [C, TQ], F32, kind="ExternalOutput")
    attn_d = nc.dram_tensor("attn", [TQ, TK], F32, kind="ExternalOutput")

    emit = _emit_v4 if SCHEME == "v4" else _emit
    ET = mybir.EngineType
    with tile.TileContext(nc) as tc:
        if loop_n is None:
            emit(tc, nc, q_d, k_d, v_d, wq_d, wk_d, wv_d, wvec_d, out_d, attn_d)
        else:
            # timing mode: run the whole (idempotent) body loop_n times on-device
            with tc.For_i(0, loop_n, 1,
                          hint_engines=(ET.PE, ET.DVE, ET.Activation, ET.SP)):
                emit(tc, nc, q_d, k_d, v_d, wq_d, wk_d, wv_d, wvec_d, out_d, attn_d)
    nc.compile()
    return nc


_NC = None


def _get_nc():
    global _NC
    if _NC is None:
        _NC = build_nc()
    return _NC


def make_in_maps(queries, keys, values, Wq, Wk, Wv, Wvec):
    if SCHEME == "v4":
        wv32 = np.ascontiguousarray(
            np.repeat(np.asarray(Wvec, dtype=np.float32).reshape(C, 1), 32, axis=1))
        wv_item = ("wvec32", wv32)
    else:
        wv_item = ("wvec16", np.ascontiguousarray(
            np.asarray(Wvec).astype(np.float16).reshape(C, 1)))
    wq = np.ascontiguousarray(np.asarray(Wq, dtype=np.float32))
    wk = np.ascontiguousarray(np.asarray(Wk, dtype=np.float32))
    wv = np.ascontiguousarray(np.asarray(Wv, dtype=np.float32))
    in_maps = []
    for b in range(B):
        in_maps.append({
            "queries": np.ascontiguousarray(np.asarray(queries)[:, b, :], dtype=np.float32),
            "keys": np.ascontiguousarray(np.asarray(keys)[:, b, :], dtype=np.float32),
            "values": np.ascontiguousarray(np.asarray(values)[:, b, :], dtype=np.float32),
            "Wq": wq, "Wk": wk, "Wv": wv, wv_item[0]: wv_item[1],
        })
    return in_maps


class _Runner:
    """Persistent sharded executable: jit once, reuse across kernel() calls."""

    def __init__(self, nc):
        import jax
        from jax.sharding import Mesh, PartitionSpec
        from jax.experimental.shard_map import shard_map
        from concourse.bass2jax import (_bass_exec_p, install_neuronx_cc_hook,
                                        partition_id_tensor)
        from concourse import mybir as _mb
        install_neuronx_cc_hook()
        self.jax = jax
        partition_name = nc.partition_id_tensor.name if nc.partition_id_tensor else None
        in_names, out_names, out_avals = [], [], []
        for alloc in nc.m.functions[0].allocations:
            if not isinstance(alloc, _mb.MemoryLocationSet):
                continue
            name = alloc.memorylocations[0].name
            if alloc.kind == "ExternalInput":
                if name != partition_name:
                    in_names.append(name)
            elif alloc.kind == "ExternalOutput":
                out_names.append(name)
                out_avals.append(jax.core.ShapedArray(
                    tuple(alloc.tensor_shape), _mb.dt.np(alloc.dtype)))
        self.in_names, self.out_names, self.out_avals = in_names, out_names, out_avals
        all_in_names = in_names + out_names
        if partition_name is not None:
            all_in_names.append(partition_name)

        def _body(*args):
            operands = list(args)
            if partition_name is not None:
                operands.append(partition_id_tensor())
            outs = _bass_exec_p.bind(
                *operands,
                out_avals=tuple(out_avals),
                in_names=tuple(all_in_names),
                out_names=tuple(out_names),
                lowering_input_output_aliases=(),
                sim_require_finite=True,
                sim_require_nnan=True,
                nc=nc,
            )
            return tuple(outs)

        devices = jax.devices()[:N_CORES]
        mesh = Mesh(np.asarray(devices), ("core",))
        in_specs = (PartitionSpec("core"),) * (len(in_names) + len(out_names))
        out_specs = (PartitionSpec("core"),) * len(out_names)
        self.fn = jax.jit(shard_map(_body, mesh=mesh, in_specs=in_specs,
                                    out_specs=out_specs, check_rep=False),
                          keep_unused=True)

    def run(self, in_maps):
        concat_in = [np.concatenate([in_maps[c][n] for c in range(N_CORES)], axis=0)
                     for n in self.in_names]
        concat_zeros = [np.zeros((N_CORES * a.shape[0], *a.shape[1:]), a.dtype)
                        for a in self.out_avals]
        outs = self.fn(*concat_in, *concat_zeros)
        res = {}
        for i, name in enumerate(self.out_names):
            res[name] = np.asarray(outs[i]).reshape(
                N_CORES, *self.out_avals[i].shape)
        return res


_RUNNER = None


def _get_runner():
    global _RUNNER
    if _RUNNER is None:
        _RUNNER = _Runner(_get_nc())
    return _RUNNER


def kernel(queries, keys, values, Wq, Wk, Wv, Wvec):
    r = _get_runner()
    in_maps = make_in_maps(queries, keys, values, Wq, Wk, Wv, Wvec)
    res = r.run(in_maps)
    out = np.ascontiguousarray(res["out"])    # (B, C, TQ)
    attn = np.ascontiguousarray(res["attn"])  # (B, TQ, TK)
    return out, attn
